# revision 12
# baseline (speedup 1.0000x reference)
"""Embedding-bag (masked mean over gathered rows) + linear head, on 8 trn2 cores.

Two-phase strategy:

Phase A (build, sharded): P[t] = emb[t] @ W.T + b, a [V, 20] f32 table,
packed 6 rows per 512-byte "super-row": phase i = tokens [i*SR, (i+1)*SR)
occupies cols [20i, 20i+20) of table[t % SR]. SR = 17408 super-rows so the
int16 gather index covers the whole vocab in a single range. Each core
projects 1/8 of the table (host supplies emb pre-transposed, so the PE
needs no on-chip transposes); the chunks are reassembled on the host
between launches.

Phase B (gather): batch rows are sorted by length (striped across cores
for balance) and pinned row->partition. gpsimd dma_gather fetches each
valid token's 512-byte super-row (single index range, ~8.4 ns/row HW
floor), DVE applies per-phase masked weights (mask * 1/len folded in,
zero weight for the 5 junk sub-rows and padding slots) and reduces over
slots. Since the projection and bias live in the table (sum of weights
is 1), the reduced [128, 20] tiles are the final outputs.
"""

import sys

sys.path.insert(0, "/opt/trn_rl_repo")

from contextlib import ExitStack

import numpy as np

import concourse.bacc as bacc
import concourse.mybir as mybir
import concourse.tile as tile
from concourse.bass_utils import run_bass_kernel_spmd

B, L = 8192, 100
V, D = 100000, 128
N_LABELS = 20
N_CORES = 8
BS = B // N_CORES          # batch rows per core
P = 128                    # partitions
N_GROUPS = BS // P         # 8 groups of 128 rows per core
SR = 17408                 # super-rows in the packed table (6 phases cover V)
PH = 6                     # phases (sub-rows per super-row)
SHARD = SR // N_CORES      # 2176 super-rows built per core
NBLK = SHARD // P          # 17 blocks of 128 super-rows per core
SCH = 24                   # gather slots per dma_gather call


# --------------------------------------------------------------- phase A
def build_program_A():
    nc = bacc.Bacc()
    # per-core pre-sliced embT phase ranges: [128 d, PH * SHARD] (zero-padded
    # where the phase range exceeds V)
    et_d = nc.declare_dram_parameter("et", [D, PH * SHARD], mybir.dt.float32, isOutput=False)
    wtb_d = nc.declare_dram_parameter("wtb", [D, N_LABELS], mybir.dt.float32, isOutput=False)
    bias_d = nc.declare_dram_parameter("bias", [1, N_LABELS], mybir.dt.float32, isOutput=False)
    chunk_d = nc.declare_dram_parameter("chunk", [SHARD, P], mybir.dt.float32, isOutput=True)

    with tile.TileContext(nc) as tc, ExitStack() as ctx:
        const_p = ctx.enter_context(tc.tile_pool(name="const", bufs=1))
        et_p = ctx.enter_context(tc.tile_pool(name="et", bufs=2))
        pk_p = ctx.enter_context(tc.tile_pool(name="pk", bufs=3))
        psum_p = ctx.enter_context(tc.tile_pool(name="ps", bufs=4, space="PSUM"))

        wtb_t = const_p.tile([D, N_LABELS], mybir.dt.float32)
        nc.sync.dma_start(out=wtb_t[:], in_=wtb_d[:])
        bias_t = const_p.tile([1, N_LABELS], mybir.dt.float32)
        nc.sync.dma_start(out=bias_t[:], in_=bias_d[:])
        ones_t = const_p.tile([1, P], mybir.dt.float32)
        nc.vector.memset(ones_t[:], 1.0)

        et_t = et_p.tile([D, PH * SHARD], mybir.dt.float32)
        nc.sync.dma_start(out=et_t[:], in_=et_d[:])

        for blk in range(NBLK):
            packed = pk_p.tile([P, P], mybir.dt.float32, tag="pk")
            nc.vector.memset(packed[:, PH * N_LABELS:], 0.0)
            for i in range(PH):
                proj = psum_p.tile([P, N_LABELS], mybir.dt.float32, tag="proj")
                nc.tensor.matmul(
                    out=proj[:], lhsT=ones_t[:1, :], rhs=bias_t[:1, :],
                    start=True, stop=False,
                )
                nc.tensor.matmul(
                    out=proj[:],
                    lhsT=et_t[:, i * SHARD + blk * P: i * SHARD + (blk + 1) * P],
                    rhs=wtb_t[:],
                    start=False, stop=True,
                )
                nc.vector.tensor_copy(
                    out=packed[:, i * N_LABELS:(i + 1) * N_LABELS], in_=proj[:]
                )
            nc.sync.dma_start(
                out=chunk_d[blk * P:(blk + 1) * P, :], in_=packed[:]
            )
    nc.compile()
    return nc


# --------------------------------------------------------------- phase B
def build_program_B(S_table, n_idx_cols, n_w_cols):
    chunks = [
        [min(SCH, int(S_table[g]) - c0) for c0 in range(0, int(S_table[g]), SCH)]
        for g in range(N_GROUPS)
    ]
    nc = bacc.Bacc()
    idx_d = nc.declare_dram_parameter("idx", [P, n_idx_cols], mybir.dt.int16, isOutput=False)
    w_d = nc.declare_dram_parameter("w", [P, n_w_cols], mybir.dt.float32, isOutput=False)
    tab_d = nc.declare_dram_parameter("tab", [SR, P], mybir.dt.float32, isOutput=False)
    out_d = nc.declare_dram_parameter("out", [BS, N_LABELS], mybir.dt.float32, isOutput=True)

    with tile.TileContext(nc) as tc, ExitStack() as ctx:
        const_p = ctx.enter_context(tc.tile_pool(name="const", bufs=1))
        g_p = ctx.enter_context(tc.tile_pool(name="gath", bufs=4))
        tmp_p = ctx.enter_context(tc.tile_pool(name="tmp", bufs=2))
        pt_p = ctx.enter_context(tc.tile_pool(name="pt", bufs=2))
        outsb_p = ctx.enter_context(tc.tile_pool(name="outsb", bufs=2))

        idx_t = const_p.tile([P, n_idx_cols], mybir.dt.int16)
        nc.sync.dma_start(out=idx_t[:], in_=idx_d[:])
        w_t = const_p.tile([P, n_w_cols], mybir.dt.float32)
        nc.sync.dma_start(out=w_t[:], in_=w_d[:])

        icol = 0
        wcol = 0
        for g in range(N_GROUPS):
            nch = len(chunks[g])
            ptile = pt_p.tile([P, N_LABELS, max(nch, 1)], mybir.dt.float32, tag="pt")
            for ci, sch in enumerate(chunks[g]):
                n = sch * P
                gath = g_p.tile([P, sch, P], mybir.dt.float32, tag="gath")
                nc.gpsimd.dma_gather(
                    out_ap=gath[:],
                    in_ap=tab_d[:],
                    idxs_ap=idx_t[:, icol:icol + 8 * sch],
                    num_idxs=n,
                    num_idxs_reg=n,
                    elem_size=P,
                    single_packet=False,
                )
                icol += 8 * sch
                # tmp[p, j, q, s] = gath[p, s, 20q+j] * w[p, (q, s)]
                tmp = tmp_p.tile([P, N_LABELS, PH, sch], mybir.dt.float32, tag="tmp")
                for q in range(PH):
                    wq = w_t[:, wcol + q * sch: wcol + (q + 1) * sch]
                    nc.vector.tensor_tensor(
                        out=tmp[:, :, q, :].transpose([0, 2, 1]),
                        in0=gath[:, :, q * N_LABELS:(q + 1) * N_LABELS],
                        in1=wq.unsqueeze(2).broadcast_to([P, sch, N_LABELS]),
                        op=mybir.AluOpType.mult,
                    )
                wcol += PH * sch
                # partial[p, j] = sum_{q,s} tmp -> ptile[:, :, ci]
                nc.vector.tensor_reduce(
                    out=ptile[:, :, ci],
                    in_=tmp[:].rearrange("p j q s -> p j (q s)"),
                    axis=mybir.AxisListType.X,
                    op=mybir.AluOpType.add,
                )
            out_sb = outsb_p.tile([P, N_LABELS], mybir.dt.float32)
            if nch > 1:
                nc.vector.tensor_reduce(
                    out=out_sb[:], in_=ptile[:],
                    axis=mybir.AxisListType.X, op=mybir.AluOpType.add,
                )
            else:
                nc.vector.tensor_copy(out=out_sb[:], in_=ptile[:, :, 0])
            nc.sync.dma_start(out=out_d[g * P:(g + 1) * P, :], in_=out_sb[:])
    nc.compile()
    return nc


_CACHE = {}


def _get_program_A():
    if "A" not in _CACHE:
        _CACHE["A"] = build_program_A()
    return _CACHE["A"]


def _get_program_B(S_table, n_idx_cols, n_w_cols):
    key = ("B", tuple(int(s) for s in S_table), n_idx_cols, n_w_cols)
    if key not in _CACHE:
        _CACHE[key] = build_program_B(S_table, n_idx_cols, n_w_cols)
    return _CACHE[key]


# --------------------------------------------------------------- host prep
def prepare_build_maps(emb, W, b):
    embT = np.ascontiguousarray(emb.T)               # [128, V]
    wtb = np.ascontiguousarray(W.T)                  # [128, 20]
    bias = np.ascontiguousarray(b[None, :])          # [1, 20]
    maps = []
    for c in range(N_CORES):
        et = np.zeros((D, PH * SHARD), dtype=np.float32)
        for i in range(PH):
            lo = i * SR + c * SHARD
            hi = min(lo + SHARD, V)
            if hi > lo:
                et[:, i * SHARD: i * SHARD + (hi - lo)] = embT[:, lo:hi]
        maps.append({"et": et, "wtb": wtb, "bias": bias})
    return maps


def plan_shards(lengths):
    rank = np.argsort(-lengths, kind="stable")
    order = np.stack([rank[c::N_CORES] for c in range(N_CORES)])  # [8, 1024]
    return order


def prepare_gather_maps(tokens, lengths, order):
    inv_len = (1.0 / lengths.astype(np.float32))
    # S per group: max length within the group's rows across all cores
    lens_c = lengths[order]                          # [8, 1024]
    S_table = np.array(
        [
            max(int(lens_c[c, g * P:(g + 1) * P].max()) for c in range(N_CORES))
            for g in range(N_GROUPS)
        ],
        dtype=np.int64,
    )
    chunks = [
        [min(SCH, int(S_table[g]) - c0) for c0 in range(0, int(S_table[g]), SCH)]
        for g in range(N_GROUPS)
    ]
    idx_maps, w_maps = [], []
    for c in range(N_CORES):
        idx_cols, w_cols = [], []
        for g in range(N_GROUPS):
            rows = order[c, g * P:(g + 1) * P]       # [128]
            S = int(S_table[g])
            tok = tokens[rows, :S]                   # [128, S]
            lens = lengths[rows]                     # [128]
            valid = np.arange(S)[None, :] < lens[:, None]
            s_idx = (tok % SR).astype(np.int16)
            s_idx[~valid] = 0
            phase = tok // SR                        # [128, S]
            wv = inv_len[rows][:, None] * valid      # [128, S] f32
            c0 = 0
            for sch in chunks[g]:
                blk = s_idx[:, c0:c0 + sch]          # [128, sch]
                # wrap for dma_gather: logical i = s*128+p -> [i%16, i//16]
                flat = blk.T.reshape(-1)
                wrapped = flat.reshape(-1, 16).T     # [16, 8*sch]
                idx_cols.append(np.tile(wrapped, (8, 1)).astype(np.int16))
                wblk = np.zeros((P, PH, sch), dtype=np.float32)
                ph = phase[:, c0:c0 + sch]
                wv_b = wv[:, c0:c0 + sch]
                for q in range(PH):
                    wblk[:, q, :] = wv_b * (ph == q)
                w_cols.append(wblk.reshape(P, PH * sch))
                c0 += sch
        idx_maps.append(np.ascontiguousarray(np.concatenate(idx_cols, axis=1)))
        w_maps.append(np.ascontiguousarray(np.concatenate(w_cols, axis=1)))
    return S_table, idx_maps, w_maps


def run_build(emb, W, b):
    nc = _get_program_A()
    maps = prepare_build_maps(emb, W, b)
    res = run_bass_kernel_spmd(nc, maps, core_ids=list(range(N_CORES)))
    table = np.concatenate(
        [res.results[c]["chunk"] for c in range(N_CORES)], axis=0
    )                                                # [SR, 128]
    return np.ascontiguousarray(table)


def run_gather(table, tokens, lengths, order, trace=False, tmpdir=None):
    S_table, idx_maps, w_maps = prepare_gather_maps(tokens, lengths, order)
    nc = _get_program_B(S_table, idx_maps[0].shape[1], w_maps[0].shape[1])
    in_maps = [
        {"idx": idx_maps[c], "w": w_maps[c], "tab": table} for c in range(N_CORES)
    ]
    res = run_bass_kernel_spmd(
        nc, in_maps, core_ids=list(range(N_CORES)), trace=trace, tmpdir=tmpdir
    )
    out = np.empty((B, N_LABELS), dtype=np.float32)
    for c in range(N_CORES):
        out[order[c]] = res.results[c]["out"]
    return out, res


def kernel(tokens, lengths, emb, W, b):
    tokens = np.ascontiguousarray(np.asarray(tokens, dtype=np.int32))
    lengths = np.asarray(lengths, dtype=np.int32)
    emb = np.ascontiguousarray(np.asarray(emb, dtype=np.float32))
    W = np.asarray(W, dtype=np.float32)
    b = np.asarray(b, dtype=np.float32)
    table = run_build(emb, W, b)
    order = plan_shards(lengths)
    out, _ = run_gather(table, tokens, lengths, order)
    return out


if __name__ == "__main__":
    rng = np.random.default_rng(0)
    toks = rng.integers(0, V, size=(B, L), dtype=np.int32)
    lens = rng.integers(1, L + 1, size=(B,), dtype=np.int32)
    emb_ = rng.standard_normal((V, D), dtype=np.float32) * 0.01
    W_ = rng.standard_normal((N_LABELS, D), dtype=np.float32) * 0.01
    b_ = rng.standard_normal((N_LABELS,), dtype=np.float32) * 0.01
    out = kernel(toks, lens, emb_, W_, b_)

    g = emb_[toks]
    mask = np.arange(L)[None, :] < lens[:, None]
    summed = np.einsum("bld,bl->bd", g, mask.astype(np.float32))
    exp = (summed / lens[:, None]) @ W_.T + b_
    err = np.abs(out - exp).max() / np.abs(exp).max()
    print("self-check rel err:", err)
